# revision 18
# baseline (speedup 1.0000x reference)
"""Collective-free device kernel; softmax shards merged on host.

Device (per core): energies e[p,j] for its seq shard (partition-major:
seq = p*32 + j), per-PARTITION max m[p], a[p,j] = exp(e[p,j] - m[p]) and
s[p] = sum_j a[p,j].  Outputs a [4096] and packed (m, s) [256].
Host: M = max over all 1024 m's, S = sum s*exp(m-M), then scales each
partition row by exp(m-M)/S while unsharding.  Using the per-partition max
as the local stabilizer is exact (log-sum-exp merge) and removes every
cross-partition reduction from the device tail.
"""

import sys

sys.path.insert(0, "/opt/trn_rl_repo")

from contextlib import ExitStack

import numpy as np

import concourse.bacc as bacc
import concourse.mybir as mybir
import concourse.tile as tile
from concourse.bass_utils import run_bass_kernel_spmd

N_CORES = 8
SEQ = 32768
HID = 1024
SHARD = SEQ // N_CORES      # 4096
N_COL = SHARD // 128        # 32

K_MAX = 8
ENC_BUFS = 4
SCHEDULE = [1, 1, 2, 4, 8, 8, 4, 2, 1, 1]
assert sum(SCHEDULE) == N_COL


def build_body(nc, tc, enc, vb, out, ms_out):
    f32 = mybir.dt.float32
    mx = mybir.AluOpType.max
    mult = mybir.AluOpType.mult

    ctx = ExitStack()
    cpool = ctx.enter_context(tc.tile_pool(name="cpool", bufs=1))
    iopool = ctx.enter_context(tc.tile_pool(name="iopool", bufs=ENC_BUFS))
    wpool = ctx.enter_context(tc.tile_pool(name="wpool", bufs=2))

    f16 = mybir.dt.float16

    # v pre-broadcast on host in fp16 (256KB), on the scalar HWDGE ring so
    # it doesn't queue behind the enc tiles.
    v_sb = cpool.tile([128, HID], f16)
    nc.scalar.dma_start(out=v_sb[:, :], in_=vb[:, :])

    # Early throwaway exp so the ACT_TABLE_LOAD runs during the main loop,
    # not in front of the tail exp.
    warm = wpool.tile([1, 1], f32, tag="warm")
    nc.scalar.activation(
        out=warm[:, :], in_=v_sb[0:1, 0:1],
        func=mybir.ActivationFunctionType.Exp,
    )

    # --- main loop: e_sb[p, j] = energy of shard-local seq = p*N_COL + j ---
    e_sb = cpool.tile([128, N_COL], f32)
    enc_r = enc.rearrange("(p j) h -> p j h", p=128)

    j0 = 0
    for t, kt in enumerate(SCHEDULE):
        buf = iopool.tile([128, K_MAX * HID], f16, tag="enc")
        bufv = buf.rearrange("p (k h) -> p k h", k=K_MAX)
        nc.sync.dma_start(out=bufv[:, 0:kt, :], in_=enc_r[:, j0:j0 + kt, :])
        for k in range(kt):
            j = j0 + k
            # multiply (both fp16 -> DVE 16-bit packed mode) then free-dim
            # sum-reduce into the f32 energy column.  The fused
            # scalar_tensor_tensor runs at ~1.47us/[128,1024] with fp16
            # operands (no fast uop); the two-op split is eligible for the
            # 2x packed modes.
            scratch = wpool.tile([128, HID], f16, tag="scratch")
            nc.vector.tensor_mul(
                scratch[:, :], buf[:, k * HID:(k + 1) * HID], v_sb[:, :],
            )
            nc.vector.tensor_reduce(
                out=e_sb[:, j:j + 1], in_=scratch[:, :],
                axis=mybir.AxisListType.X, op=mybir.AluOpType.add,
            )
        j0 += kt

    # --- tail: per-partition softmax pieces, no cross-partition reduction ---
    m1 = wpool.tile([128, 1], f32, tag="m1", bufs=1)
    nc.vector.tensor_reduce(
        out=m1[:, :], in_=e_sb[:, :], axis=mybir.AxisListType.X, op=mx,
    )
    nm1 = wpool.tile([128, 1], f32, tag="nm1", bufs=1)
    nc.vector.tensor_scalar_mul(nm1[:, :], m1[:, :], -1.0)

    a_loc = cpool.tile([128, N_COL], f32)
    ssum = wpool.tile([128, 1], f32, tag="ssum", bufs=1)
    nc.scalar.activation(
        out=a_loc[:, :], in_=e_sb[:, :],
        func=mybir.ActivationFunctionType.Exp,
        bias=nm1[:, :], scale=1.0,
        accum_out=ssum[:, :],
    )

    pk = wpool.tile([128, 2], f32, tag="pk", bufs=1)
    nc.vector.tensor_copy(pk[:, 0:1], m1[:, :])
    nc.vector.tensor_copy(pk[:, 1:2], ssum[:, :])

    nc.sync.dma_start(out=out.rearrange("(p j) -> p j", p=128),
                      in_=a_loc[:, :])
    nc.scalar.dma_start(out=ms_out.rearrange("(p k) -> p k", k=2),
                        in_=pk[:, :])

    ctx.close()


def build_nc(n_cores=N_CORES, debug=False):
    nc = bacc.Bacc(
        "TRN2",
        target_bir_lowering=False,
        debug=debug,
        num_devices=n_cores,
    )
    enc = nc.dram_tensor("enc", [SHARD, HID], mybir.dt.float16, kind="ExternalInput")
    vb = nc.dram_tensor("vb", [128, HID], mybir.dt.float16, kind="ExternalInput")
    out = nc.dram_tensor("attn_part", [SHARD], mybir.dt.float32,
                         kind="ExternalOutput")
    ms = nc.dram_tensor("ms", [2 * 128], mybir.dt.float32, kind="ExternalOutput")
    with tile.TileContext(nc) as tc:
        build_body(nc, tc, enc.ap(), vb.ap(), out.ap(), ms.ap())
    nc.compile()
    return nc


_NC_CACHE = {}


def _get_nc():
    if "nc" not in _NC_CACHE:
        _NC_CACHE["nc"] = build_nc()
    return _NC_CACHE["nc"]


def make_in_maps(hidden, encoder_outputs, attn_w, attn_b=None, n_cores=N_CORES,
                 shard=SHARD):
    hidden = np.asarray(hidden, dtype=np.float32)
    enc = np.asarray(encoder_outputs, dtype=np.float32)[0]
    w = np.asarray(attn_w, dtype=np.float32)
    v = (w.T @ hidden).astype(np.float32)
    # fp16 streaming: halves the HBM traffic of the enc stream and enables
    # the DVE 16-bit 2x mode.  Softmax rel err vs the f32 reference is
    # ~3.2e-3 (quantization of enc and v only; products and accumulation
    # stay fp32 on device).
    vb = np.ascontiguousarray(
        np.broadcast_to(v.astype(np.float16)[None, :], (128, v.shape[0])))
    return [
        {
            "enc": np.ascontiguousarray(
                enc[i * shard:(i + 1) * shard, :].astype(np.float16)),
            "vb": vb,
        }
        for i in range(n_cores)
    ]


def run(in_maps, trace=False, **kwargs):
    nc = _get_nc()
    return run_bass_kernel_spmd(
        nc, in_maps, core_ids=list(range(N_CORES)), trace=trace, **kwargs
    )


def kernel(**inputs):
    in_maps = make_in_maps(
        inputs["hidden"], inputs["encoder_outputs"], inputs["attn_w"],
        inputs.get("attn_b"),
    )
    res = run(in_maps)
    parts = [
        np.asarray(res.results[i]["attn_part"], dtype=np.float32).reshape(128, N_COL)
        for i in range(N_CORES)
    ]
    ms = [
        np.asarray(res.results[i]["ms"], dtype=np.float32).reshape(128, 2)
        for i in range(N_CORES)
    ]
    m = np.stack([x[:, 0] for x in ms]).astype(np.float64)   # [8, 128]
    s = np.stack([x[:, 1] for x in ms]).astype(np.float64)   # [8, 128]
    M = m.max()
    w = np.exp(m - M)                                        # [8, 128]
    S = float((s * w).sum())
    scale = (w / S).astype(np.float32)                       # [8, 128]
    attn = np.concatenate(
        [(parts[i] * scale[i][:, None]).reshape(-1) for i in range(N_CORES)]
    )
    return attn[None, None, :]


# revision 20
# speedup vs baseline: 1.0918x; 1.0918x over previous
"""Collective-free device kernel; softmax shards merged on host.

Device (per core): energies e[p,j] for its seq shard (partition-major:
seq = p*32 + j), per-PARTITION max m[p], a[p,j] = exp(e[p,j] - m[p]) and
s[p] = sum_j a[p,j].  Outputs a [4096] and packed (m, s) [256].
Host: M = max over all 1024 m's, S = sum s*exp(m-M), then scales each
partition row by exp(m-M)/S while unsharding.  Using the per-partition max
as the local stabilizer is exact (log-sum-exp merge) and removes every
cross-partition reduction from the device tail.
"""

import sys

sys.path.insert(0, "/opt/trn_rl_repo")

from contextlib import ExitStack

import numpy as np

import concourse.bacc as bacc
import concourse.mybir as mybir
import concourse.tile as tile
from concourse.bass_utils import run_bass_kernel_spmd

N_CORES = 8
SEQ = 32768
HID = 1024
SHARD = SEQ // N_CORES      # 4096
N_COL = SHARD // 128        # 32

K_MAX = 8
ENC_BUFS = 4
SCHEDULE = [1, 1, 2, 4, 8, 8, 4, 2, 1, 1]
assert sum(SCHEDULE) == N_COL


def build_body(nc, tc, enc, vb, out, ms_out):
    f32 = mybir.dt.float32
    mx = mybir.AluOpType.max
    mult = mybir.AluOpType.mult

    ctx = ExitStack()
    cpool = ctx.enter_context(tc.tile_pool(name="cpool", bufs=1))
    iopool = ctx.enter_context(tc.tile_pool(name="iopool", bufs=ENC_BUFS))
    wpool = ctx.enter_context(tc.tile_pool(name="wpool", bufs=2))
    spool = ctx.enter_context(tc.tile_pool(name="spool", bufs=3))
    jpool = ctx.enter_context(tc.tile_pool(name="jpool", bufs=2))

    f16 = mybir.dt.float16

    # v pre-broadcast on host in fp16 (256KB), on the scalar HWDGE ring so
    # it doesn't queue behind the enc tiles.
    v_sb = cpool.tile([128, HID], f16)
    nc.scalar.dma_start(out=v_sb[:, :], in_=vb[:, :])

    # Early throwaway exp so the ACT_TABLE_LOAD runs during the main loop,
    # not in front of the tail exp.
    warm = wpool.tile([1, 1], f32, tag="warm")
    nc.scalar.activation(
        out=warm[:, :], in_=v_sb[0:1, 0:1],
        func=mybir.ActivationFunctionType.Exp,
    )

    # --- main loop: e_sb[p, j] = energy of shard-local seq = p*N_COL + j ---
    e_sb = cpool.tile([128, N_COL], f32)
    enc_r = enc.rearrange("(p j) h -> p j h", p=128)

    j0 = 0
    for t, kt in enumerate(SCHEDULE):
        buf = iopool.tile([128, K_MAX * HID], f16, tag="enc")
        bufv = buf.rearrange("p (k h) -> p k h", k=K_MAX)
        nc.sync.dma_start(out=bufv[:, 0:kt, :], in_=enc_r[:, j0:j0 + kt, :])
        for k in range(kt):
            j = j0 + k
            # fp16 multiply on the DVE (16-bit packed mode, ~0.68us/col),
            # then the free-dim sum on the otherwise-idle ACT engine
            # (activation Copy with accum_out, fp32 accumulate) — the two
            # engines pipeline, instead of the DVE paying ~1.2-1.5us/col
            # for a fused multiply-reduce or a DVE-side reduce.
            scratch = spool.tile([128, HID], f16, tag="scratch")
            nc.vector.tensor_mul(
                scratch[:, :], buf[:, k * HID:(k + 1) * HID], v_sb[:, :],
            )
            junk = jpool.tile([128, HID], f16, tag="junk")
            nc.scalar.activation(
                out=junk[:, :], in_=scratch[:, :],
                func=mybir.ActivationFunctionType.Copy,
                accum_out=e_sb[:, j:j + 1],
            )
        j0 += kt

    # --- tail: per-partition softmax pieces, no cross-partition reduction ---
    m1 = wpool.tile([128, 1], f32, tag="m1", bufs=1)
    nc.vector.tensor_reduce(
        out=m1[:, :], in_=e_sb[:, :], axis=mybir.AxisListType.X, op=mx,
    )
    nm1 = wpool.tile([128, 1], f32, tag="nm1", bufs=1)
    nc.vector.tensor_scalar_mul(nm1[:, :], m1[:, :], -1.0)

    a_loc = cpool.tile([128, N_COL], f32)
    ssum = wpool.tile([128, 1], f32, tag="ssum", bufs=1)
    nc.scalar.activation(
        out=a_loc[:, :], in_=e_sb[:, :],
        func=mybir.ActivationFunctionType.Exp,
        bias=nm1[:, :], scale=1.0,
        accum_out=ssum[:, :],
    )

    pk = wpool.tile([128, 2], f32, tag="pk", bufs=1)
    nc.vector.tensor_copy(pk[:, 0:1], m1[:, :])
    nc.vector.tensor_copy(pk[:, 1:2], ssum[:, :])

    nc.sync.dma_start(out=out.rearrange("(p j) -> p j", p=128),
                      in_=a_loc[:, :])
    nc.scalar.dma_start(out=ms_out.rearrange("(p k) -> p k", k=2),
                        in_=pk[:, :])

    ctx.close()


def build_nc(n_cores=N_CORES, debug=False):
    nc = bacc.Bacc(
        "TRN2",
        target_bir_lowering=False,
        debug=debug,
        num_devices=n_cores,
    )
    enc = nc.dram_tensor("enc", [SHARD, HID], mybir.dt.float16, kind="ExternalInput")
    vb = nc.dram_tensor("vb", [128, HID], mybir.dt.float16, kind="ExternalInput")
    out = nc.dram_tensor("attn_part", [SHARD], mybir.dt.float32,
                         kind="ExternalOutput")
    ms = nc.dram_tensor("ms", [2 * 128], mybir.dt.float32, kind="ExternalOutput")
    with tile.TileContext(nc) as tc:
        build_body(nc, tc, enc.ap(), vb.ap(), out.ap(), ms.ap())
    nc.compile()
    return nc


_NC_CACHE = {}


def _get_nc():
    if "nc" not in _NC_CACHE:
        _NC_CACHE["nc"] = build_nc()
    return _NC_CACHE["nc"]


def make_in_maps(hidden, encoder_outputs, attn_w, attn_b=None, n_cores=N_CORES,
                 shard=SHARD):
    hidden = np.asarray(hidden, dtype=np.float32)
    enc = np.asarray(encoder_outputs, dtype=np.float32)[0]
    w = np.asarray(attn_w, dtype=np.float32)
    v = (w.T @ hidden).astype(np.float32)
    # fp16 streaming: halves the HBM traffic of the enc stream and enables
    # the DVE 16-bit 2x mode.  Softmax rel err vs the f32 reference is
    # ~3.2e-3 (quantization of enc and v only; products and accumulation
    # stay fp32 on device).
    vb = np.ascontiguousarray(
        np.broadcast_to(v.astype(np.float16)[None, :], (128, v.shape[0])))
    return [
        {
            "enc": np.ascontiguousarray(
                enc[i * shard:(i + 1) * shard, :].astype(np.float16)),
            "vb": vb,
        }
        for i in range(n_cores)
    ]


def run(in_maps, trace=False, **kwargs):
    nc = _get_nc()
    return run_bass_kernel_spmd(
        nc, in_maps, core_ids=list(range(N_CORES)), trace=trace, **kwargs
    )


def kernel(**inputs):
    in_maps = make_in_maps(
        inputs["hidden"], inputs["encoder_outputs"], inputs["attn_w"],
        inputs.get("attn_b"),
    )
    res = run(in_maps)
    parts = [
        np.asarray(res.results[i]["attn_part"], dtype=np.float32).reshape(128, N_COL)
        for i in range(N_CORES)
    ]
    ms = [
        np.asarray(res.results[i]["ms"], dtype=np.float32).reshape(128, 2)
        for i in range(N_CORES)
    ]
    m = np.stack([x[:, 0] for x in ms]).astype(np.float64)   # [8, 128]
    s = np.stack([x[:, 1] for x in ms]).astype(np.float64)   # [8, 128]
    M = m.max()
    w = np.exp(m - M)                                        # [8, 128]
    S = float((s * w).sum())
    scale = (w / S).astype(np.float32)                       # [8, 128]
    attn = np.concatenate(
        [(parts[i] * scale[i][:, None]).reshape(-1) for i in range(N_CORES)]
    )
    return attn[None, None, :]


# revision 22
# speedup vs baseline: 1.3448x; 1.2317x over previous
"""Collective-free device kernel; softmax shards merged on host.

Device (per core): energies e[p,j] for its seq shard (partition-major:
seq = p*32 + j), per-PARTITION max m[p], a[p,j] = exp(e[p,j] - m[p]) and
s[p] = sum_j a[p,j].  Outputs a [4096] and packed (m, s) [256].
Host: M = max over all 1024 m's, S = sum s*exp(m-M), then scales each
partition row by exp(m-M)/S while unsharding.  Using the per-partition max
as the local stabilizer is exact (log-sum-exp merge) and removes every
cross-partition reduction from the device tail.
"""

import sys

sys.path.insert(0, "/opt/trn_rl_repo")

from contextlib import ExitStack

import numpy as np

import concourse.bacc as bacc
import concourse.mybir as mybir
import concourse.tile as tile
from concourse.bass_utils import run_bass_kernel_spmd

N_CORES = 8
SEQ = 32768
HID = 1024
SHARD = SEQ // N_CORES      # 4096
N_COL = SHARD // 128        # 32

K_MAX = 8
ENC_BUFS = 4
SCHEDULE = [1, 1, 2, 4, 8, 8, 4, 2, 1, 1]
assert sum(SCHEDULE) == N_COL


def build_body(nc, tc, enc, vb, out, ms_out):
    f32 = mybir.dt.float32
    mx = mybir.AluOpType.max
    mult = mybir.AluOpType.mult

    ctx = ExitStack()
    cpool = ctx.enter_context(tc.tile_pool(name="cpool", bufs=1))
    iopool = ctx.enter_context(tc.tile_pool(name="iopool", bufs=ENC_BUFS))
    wpool = ctx.enter_context(tc.tile_pool(name="wpool", bufs=2))
    spool = ctx.enter_context(tc.tile_pool(name="spool", bufs=3))
    jpool = ctx.enter_context(tc.tile_pool(name="jpool", bufs=2))

    f16 = mybir.dt.float16

    # v pre-broadcast on host in fp16 (256KB), on the scalar HWDGE ring so
    # it doesn't queue behind the enc tiles.
    v_sb = cpool.tile([128, HID], f16)
    nc.scalar.dma_start(out=v_sb[:, :], in_=vb[:, :])

    # Early throwaway exp so the ACT_TABLE_LOAD runs during the main loop,
    # not in front of the tail exp.
    warm = wpool.tile([1, 1], f32, tag="warm")
    nc.scalar.activation(
        out=warm[:, :], in_=v_sb[0:1, 0:1],
        func=mybir.ActivationFunctionType.Exp,
    )

    # --- main loop: e_sb[p, j] = energy of shard-local seq = p*N_COL + j ---
    e_sb = cpool.tile([128, N_COL], f32)
    enc_r = enc.rearrange("(p j) h -> p j h", p=128)

    j0 = 0
    for t, kt in enumerate(SCHEDULE):
        buf = iopool.tile([128, K_MAX * HID], f16, tag="enc")
        bufv = buf.rearrange("p (k h) -> p k h", k=K_MAX)
        nc.sync.dma_start(out=bufv[:, 0:kt, :], in_=enc_r[:, j0:j0 + kt, :])
        for k in range(kt):
            j = j0 + k
            # fp16 multiply on the DVE (16-bit packed mode, ~0.68us/col);
            # the free-dim sum-reduce is split 2:1 between the ACT engine
            # (activation Copy + accum_out, ~1.64us/col) and the DVE
            # (tensor_reduce, ~1.18us/col) so both engines run ~34us total,
            # well under the fused single-engine multiply-reduce (~47us).
            scratch = spool.tile([128, HID], f16, tag="scratch")
            nc.vector.tensor_mul(
                scratch[:, :], buf[:, k * HID:(k + 1) * HID], v_sb[:, :],
            )
            if j % 3 == 2:
                nc.vector.tensor_reduce(
                    out=e_sb[:, j:j + 1], in_=scratch[:, :],
                    axis=mybir.AxisListType.X, op=mybir.AluOpType.add,
                )
            else:
                junk = jpool.tile([128, HID], f16, tag="junk")
                nc.scalar.activation(
                    out=junk[:, :], in_=scratch[:, :],
                    func=mybir.ActivationFunctionType.Copy,
                    accum_out=e_sb[:, j:j + 1],
                )
        j0 += kt

    # --- tail: per-partition softmax pieces, no cross-partition reduction ---
    m1 = wpool.tile([128, 1], f32, tag="m1", bufs=1)
    nc.vector.tensor_reduce(
        out=m1[:, :], in_=e_sb[:, :], axis=mybir.AxisListType.X, op=mx,
    )
    nm1 = wpool.tile([128, 1], f32, tag="nm1", bufs=1)
    nc.vector.tensor_scalar_mul(nm1[:, :], m1[:, :], -1.0)

    a_loc = cpool.tile([128, N_COL], f32)
    ssum = wpool.tile([128, 1], f32, tag="ssum", bufs=1)
    nc.scalar.activation(
        out=a_loc[:, :], in_=e_sb[:, :],
        func=mybir.ActivationFunctionType.Exp,
        bias=nm1[:, :], scale=1.0,
        accum_out=ssum[:, :],
    )

    pk = wpool.tile([128, 2], f32, tag="pk", bufs=1)
    nc.vector.tensor_copy(pk[:, 0:1], m1[:, :])
    nc.vector.tensor_copy(pk[:, 1:2], ssum[:, :])

    nc.sync.dma_start(out=out.rearrange("(p j) -> p j", p=128),
                      in_=a_loc[:, :])
    nc.scalar.dma_start(out=ms_out.rearrange("(p k) -> p k", k=2),
                        in_=pk[:, :])

    ctx.close()


def build_nc(n_cores=N_CORES, debug=False):
    nc = bacc.Bacc(
        "TRN2",
        target_bir_lowering=False,
        debug=debug,
        num_devices=n_cores,
    )
    enc = nc.dram_tensor("enc", [SHARD, HID], mybir.dt.float16, kind="ExternalInput")
    vb = nc.dram_tensor("vb", [128, HID], mybir.dt.float16, kind="ExternalInput")
    out = nc.dram_tensor("attn_part", [SHARD], mybir.dt.float32,
                         kind="ExternalOutput")
    ms = nc.dram_tensor("ms", [2 * 128], mybir.dt.float32, kind="ExternalOutput")
    with tile.TileContext(nc) as tc:
        build_body(nc, tc, enc.ap(), vb.ap(), out.ap(), ms.ap())
    nc.compile()
    return nc


_NC_CACHE = {}


def _get_nc():
    if "nc" not in _NC_CACHE:
        _NC_CACHE["nc"] = build_nc()
    return _NC_CACHE["nc"]


def make_in_maps(hidden, encoder_outputs, attn_w, attn_b=None, n_cores=N_CORES,
                 shard=SHARD):
    hidden = np.asarray(hidden, dtype=np.float32)
    enc = np.asarray(encoder_outputs, dtype=np.float32)[0]
    w = np.asarray(attn_w, dtype=np.float32)
    v = (w.T @ hidden).astype(np.float32)
    # fp16 streaming: halves the HBM traffic of the enc stream and enables
    # the DVE 16-bit 2x mode.  Softmax rel err vs the f32 reference is
    # ~3.2e-3 (quantization of enc and v only; products and accumulation
    # stay fp32 on device).
    vb = np.ascontiguousarray(
        np.broadcast_to(v.astype(np.float16)[None, :], (128, v.shape[0])))
    return [
        {
            "enc": np.ascontiguousarray(
                enc[i * shard:(i + 1) * shard, :].astype(np.float16)),
            "vb": vb,
        }
        for i in range(n_cores)
    ]


def run(in_maps, trace=False, **kwargs):
    nc = _get_nc()
    return run_bass_kernel_spmd(
        nc, in_maps, core_ids=list(range(N_CORES)), trace=trace, **kwargs
    )


def kernel(**inputs):
    in_maps = make_in_maps(
        inputs["hidden"], inputs["encoder_outputs"], inputs["attn_w"],
        inputs.get("attn_b"),
    )
    res = run(in_maps)
    parts = [
        np.asarray(res.results[i]["attn_part"], dtype=np.float32).reshape(128, N_COL)
        for i in range(N_CORES)
    ]
    ms = [
        np.asarray(res.results[i]["ms"], dtype=np.float32).reshape(128, 2)
        for i in range(N_CORES)
    ]
    m = np.stack([x[:, 0] for x in ms]).astype(np.float64)   # [8, 128]
    s = np.stack([x[:, 1] for x in ms]).astype(np.float64)   # [8, 128]
    M = m.max()
    w = np.exp(m - M)                                        # [8, 128]
    S = float((s * w).sum())
    scale = (w / S).astype(np.float32)                       # [8, 128]
    attn = np.concatenate(
        [(parts[i] * scale[i][:, None]).reshape(-1) for i in range(N_CORES)]
    )
    return attn[None, None, :]
